# revision 6
# baseline (speedup 1.0000x reference)
"""Trainium2 Bass kernel for the video-adapter module.

Computation (per clip of T=8 frames, H=W=14, C=768, CA=384):
  h  = fc1(x[:, 1:, :])                    # 768 -> 384
  h  = depthwise_conv3d(h, 3x3x3, pad 1)   # per-channel over (T, H, W)
  h  = fc2(h)                              # 384 -> 768
  out = x;  out[:, 1:, :] += h

Sharding: data-parallel over the clip axis — 8 cores x 4 clips each.
Inputs are replicated weights + a per-core x shard; no collectives.

Per-core dataflow (all token indices are clip-local, CLS rows kept
interleaved so every DMA/matmul tile is a clean 128-row block):
  DMA x (bf16, host-converted)  -> PE transpose -> xT (C-major)
  fc1 matmuls (W1T stationary)  -> h channel-major [128ch, 1576tok]
  depthwise conv = 27 fused scalar_tensor_tensor FMA taps (DVE/GPSIMD)
      acc += shift(h) * w[ch, tap]   with clipped valid ranges
  fc2 matmuls (acc slices stationary) -> psum token-major
  residual: out = psum + x (fused STT)  -> DMA out (f32)
"""

import os
import sys

sys.path.insert(0, "/opt/trn_rl_repo")

import numpy as np
import ml_dtypes

import concourse.bass as bass
import concourse.bacc as bacc
import concourse.mybir as mybir
import concourse.tile as tile
from concourse.ap import AP
from concourse.bass_utils import run_bass_kernel_spmd

BF16 = mybir.dt.bfloat16
F32 = mybir.dt.float32
NPBF16 = ml_dtypes.bfloat16
MULT = mybir.AluOpType.mult
ADD = mybir.AluOpType.add

N_CORES = 8
T = 8
L = 197            # tokens per frame incl CLS
GRID = 14          # H = W
C = 768
CA = 384
NCLIP = 4          # clips per core
CLIP_ROWS = T * L  # 1576
CORE_ROWS = NCLIP * CLIP_ROWS  # 6304
NBLK = 13          # 128-row blocks per clip (12 full + 40)
CHUNKS = [(0, 512), (512, 1024), (1024, 1536), (1536, 1576)]

# tap order: d = (dt+1)*9 + (dh+1)*3 + (dw+1); center = 13
TAPS = [(dt, dh, dw) for dt in (-1, 0, 1) for dh in (-1, 0, 1) for dw in (-1, 0, 1)]

# Engine per tap: "V" = vector (DVE), "G" = gpsimd. Tunable for balance.
TAP_ENGINE = {d: "V" for d in range(27)}

_CACHE = {}
LAST_RESULT = None
last_exec_time_ns = None


def _blk_rows(b):
    return 128 if b < NBLK - 1 else CLIP_ROWS - 128 * (NBLK - 1)


def _flat_ap(t, off, tcnt, n):
    """3-dim AP on a clip tile: partitions x (t frames, step 197) x (n flat
    tokens, step 1), starting at in-frame offset `off`."""
    full = t[:]
    return AP(full.tensor, full.offset + off, [list(full.ap[0]), [L, tcnt], [1, n]])


def _row_ap(t, off, tcnt, hcnt):
    """3-dim AP: partitions x (t frames, step 197) x (h rows, step 14)."""
    full = t[:]
    return AP(full.tensor, full.offset + off, [list(full.ap[0]), [L, tcnt], [GRID, hcnt]])


def _cls_ap(t):
    full = t[:]
    return AP(full.tensor, full.offset, [list(full.ap[0]), [L, T]])


def _build(has_b1, has_b2p, tap_engine):
    nc = bacc.Bacc("TRN2", target_bir_lowering=False, debug=False,
                   enable_asserts=False)

    x_d = nc.dram_tensor("x", [CORE_ROWS, C], BF16, kind="ExternalInput")
    w1t_d = nc.dram_tensor("w1t", [C, CA], BF16, kind="ExternalInput")
    w2t_d = nc.dram_tensor("w2t", [CA, C], BF16, kind="ExternalInput")
    cw_d = nc.dram_tensor("cw", [CA, 54], F32, kind="ExternalInput")
    id_d = nc.dram_tensor("ident", [128, 128], BF16, kind="ExternalInput")
    if has_b1:
        b1_d = nc.dram_tensor("b1", [CA, 1], F32, kind="ExternalInput")
    if has_b2p:
        b2p_d = nc.dram_tensor("b2p", [1, C], BF16, kind="ExternalInput")
        ones_d = nc.dram_tensor("ones", [1, 128], BF16, kind="ExternalInput")
    out_d = nc.dram_tensor("out", [CORE_ROWS, C], F32, kind="ExternalOutput")

    with tile.TileContext(nc) as tc:
        with (
            tc.tile_pool(name="const", bufs=1) as cp,
            tc.tile_pool(name="xbf", bufs=27) as xp,
            tc.tile_pool(name="xt", bufs=12) as xtp,
            tc.tile_pool(name="hcm", bufs=6) as hp,
            tc.tile_pool(name="acc", bufs=6) as accp,
            tc.tile_pool(name="osb", bufs=4) as op_,
            tc.tile_pool(name="pxt", bufs=2, space=bass.MemorySpace.PSUM) as pxt,
            tc.tile_pool(name="ph", bufs=2, space=bass.MemorySpace.PSUM) as php,
            tc.tile_pool(name="po", bufs=4, space=bass.MemorySpace.PSUM) as pop,
        ):
            w1 = []
            for g in range(6):
                t = cp.tile([128, CA], BF16, tag=f"w1_{g}")
                nc.sync.dma_start(t[:], w1t_d[128 * g:128 * (g + 1), :])
                w1.append(t)
            w2 = []
            for k in range(3):
                t = cp.tile([128, C], BF16, tag=f"w2_{k}")
                nc.sync.dma_start(t[:], w2t_d[128 * k:128 * (k + 1), :])
                w2.append(t)
            cw = []
            for m in range(3):
                t = cp.tile([128, 54], F32, tag=f"cw_{m}")
                nc.sync.dma_start(t[:], cw_d[128 * m:128 * (m + 1), :])
                cw.append(t)
            ident = cp.tile([128, 128], BF16, tag="ident")
            nc.sync.dma_start(ident[:], id_d[:, :])
            if has_b1:
                b1sb = []
                for m in range(3):
                    t = cp.tile([128, 1], F32, tag=f"b1_{m}")
                    nc.sync.dma_start(t[:], b1_d[128 * m:128 * (m + 1), :])
                    b1sb.append(t)
            if has_b2p:
                b2p = cp.tile([1, C], BF16, tag="b2p")
                nc.sync.dma_start(b2p[:], b2p_d[:, :])
                ones = cp.tile([1, 128], BF16, tag="ones")
                nc.sync.dma_start(ones[:], ones_d[:, :])

            for c in range(NCLIP):
                base = CLIP_ROWS * c
                # ---- load x blocks (bf16) ----
                xbf = []
                for b in range(NBLK):
                    nb = _blk_rows(b)
                    t = xp.tile([128, C], BF16, tag="xbf", name=f"xbf_{c}_{b}")
                    nc.sync.dma_start(
                        t[0:nb, :], x_d[base + 128 * b: base + 128 * b + nb, :])
                    xbf.append(t)

                # ---- transpose + fc1, chunk by chunk ----
                hcm = [hp.tile([128, CLIP_ROWS + 16], BF16, tag="hcm", name=f"hcm_{c}_{m}")
                       for m in range(3)]
                for (s0, s1) in CHUNKS:
                    w = s1 - s0
                    xt = [xtp.tile([128, 512], BF16, tag="xt", name=f"xt_{c}_{s0}_{g}")
                          for g in range(6)]
                    for g in range(6):
                        pst = pxt.tile([128, 512], BF16, tag="pxt", name=f"pxt_{c}_{s0}_{g}")
                        for bi in range(s0 // 128, (s1 + 127) // 128):
                            nb = min(_blk_rows(bi), s1 - 128 * bi)
                            nc.tensor.transpose(
                                pst[:, 128 * bi - s0:128 * bi - s0 + nb],
                                xbf[bi][0:nb, 128 * g:128 * (g + 1)],
                                ident[0:nb, 0:nb])
                        nc.scalar.copy(xt[g][:, 0:w], pst[:, 0:w])
                    for m in range(3):
                        psh = php.tile([128, 512], F32, tag="ph", name=f"ph_{c}_{s0}_{m}")
                        for g in range(6):
                            nc.tensor.matmul(
                                psh[:, 0:w],
                                w1[g][:, 128 * m:128 * (m + 1)],
                                xt[g][:, 0:w],
                                start=(g == 0), stop=(g == 5))
                        if has_b1:
                            nc.vector.tensor_scalar_add(
                                hcm[m][:, s0:s1], psh[:, 0:w], b1sb[m][:, 0:1])
                        else:
                            nc.scalar.copy(hcm[m][:, s0:s1], psh[:, 0:w])

                # ---- depthwise conv: fused FMA taps ----
                # Each tap runs on a (h,w)-flattened 3-dim AP (walrus caps
                # TensorScalarPtr at 2 free dims). For dw != 0 the flat run
                # wraps at row edges; the wrapped term is subtracted with a
                # small negated-weight fixup op (cols 27..53 of cw = -w).
                acc = [accp.tile([128, CLIP_ROWS], BF16, tag="acc", name=f"acc_{c}_{m}")
                       for m in range(3)]
                for m in range(3):
                    nc.gpsimd.memset(_cls_ap(acc[m]), 0.0)
                    nc.gpsimd.memset(hcm[m][:, CLIP_ROWS:], 0.0)
                    for d, (dt_, dh_, dw_) in enumerate(TAPS):
                        t0, h0 = max(0, -dt_), max(0, -dh_)
                        tcn, hcn = T - abs(dt_), GRID - abs(dh_)
                        out_off = L * t0 + GRID * h0 + 1
                        in_off = (L * (t0 + dt_) + GRID * (h0 + dh_)
                                  + 1 + dw_)
                        n = GRID * hcn
                        oap = _flat_ap(acc[m], out_off, tcn, n)
                        iap = _flat_ap(hcm[m], in_off, tcn, n)
                        eng = nc.vector if tap_engine[d] == "V" else nc.gpsimd
                        if d == 13:
                            # center tap initializes the accumulator
                            nc.vector.tensor_scalar_mul(oap, iap, cw[m][:, d:d + 1])
                        else:
                            eng.scalar_tensor_tensor(
                                oap, iap, cw[m][:, d:d + 1], oap,
                                op0=MULT, op1=ADD)
                        if dw_ != 0:
                            wf = GRID - 1 if dw_ == 1 else 0
                            o2 = _row_ap(acc[m], L * t0 + GRID * h0 + wf + 1,
                                         tcn, hcn)
                            i2 = _row_ap(hcm[m],
                                         L * (t0 + dt_) + GRID * (h0 + dh_)
                                         + wf + dw_ + 1, tcn, hcn)
                            eng.scalar_tensor_tensor(
                                o2, i2, cw[m][:, 27 + d:28 + d], o2,
                                op0=MULT, op1=ADD)

                # ---- fc2 + residual + store ----
                for b in range(NBLK):
                    nb = _blk_rows(b)
                    pso = [pop.tile([128, CA], F32, tag="po", name=f"po_{c}_{b}_{nh}") for nh in range(2)]
                    for nh in range(2):
                        for k in range(3):
                            nc.tensor.matmul(
                                pso[nh][0:nb, :],
                                acc[k][:, 128 * b:128 * b + nb],
                                w2[k][:, CA * nh:CA * (nh + 1)],
                                start=(k == 0),
                                stop=(k == 2 and not has_b2p))
                        if has_b2p:
                            nc.tensor.matmul(
                                pso[nh][0:nb, :],
                                ones[0:1, 0:nb],
                                b2p[0:1, CA * nh:CA * (nh + 1)],
                                start=False, stop=True)
                    osb = op_.tile([128, C], F32, tag="osb", name=f"osb_{c}_{b}")
                    for nh in range(2):
                        nc.vector.scalar_tensor_tensor(
                            osb[0:nb, CA * nh:CA * (nh + 1)],
                            pso[nh][0:nb, :], 1.0,
                            xbf[b][0:nb, CA * nh:CA * (nh + 1)],
                            op0=MULT, op1=ADD)
                    nc.sync.dma_start(
                        out_d[base + 128 * b: base + 128 * b + nb, :],
                        osb[0:nb, :])

    nc.compile()
    return nc


def _get_nc(has_b1, has_b2p):
    key = (has_b1, has_b2p, tuple(sorted(TAP_ENGINE.items())))
    if key not in _CACHE:
        _CACHE[key] = _build(has_b1, has_b2p, TAP_ENGINE)
    return _CACHE[key]


def kernel(x, W1, b1, conv_w, conv_b, W2, b2, T=8):
    global LAST_RESULT, last_exec_time_ns
    x = np.asarray(x, dtype=np.float32)
    W1 = np.asarray(W1, dtype=np.float32)
    b1 = np.asarray(b1, dtype=np.float32)
    conv_w = np.asarray(conv_w, dtype=np.float32)
    conv_b = np.asarray(conv_b, dtype=np.float32)
    W2 = np.asarray(W2, dtype=np.float32)
    b2 = np.asarray(b2, dtype=np.float32)

    xs = np.ascontiguousarray(
        x.reshape(N_CORES, CORE_ROWS, C)).astype(NPBF16)
    w1t = np.ascontiguousarray(W1.T).astype(NPBF16)
    w2t = np.ascontiguousarray(W2.T).astype(NPBF16)
    cw27 = conv_w.reshape(CA, 27).astype(np.float32)
    cwf = np.ascontiguousarray(np.concatenate([cw27, -cw27], axis=1))
    b2p = (b2.astype(np.float64)
           + W2.astype(np.float64) @ conv_b.astype(np.float64))
    has_b1 = bool(np.any(b1 != 0.0))
    has_b2p = bool(np.any(b2p != 0.0))
    ident = np.eye(128, dtype=NPBF16)

    common = {
        "w1t": w1t,
        "w2t": w2t,
        "cw": cwf,
        "ident": ident,
    }
    if has_b1:
        common["b1"] = b1.reshape(CA, 1).astype(np.float32)
    if has_b2p:
        common["b2p"] = b2p.reshape(1, C).astype(NPBF16)
        common["ones"] = np.ones((1, 128), dtype=NPBF16)

    nc = _get_nc(has_b1, has_b2p)
    in_maps = [dict(common, x=xs[i]) for i in range(N_CORES)]
    trace = bool(int(os.environ.get("KERNEL_TRACE", "0")))
    res = run_bass_kernel_spmd(nc, in_maps, list(range(N_CORES)), trace=trace)
    LAST_RESULT = res
    last_exec_time_ns = res.exec_time_ns

    out = np.stack([np.asarray(res.results[i]["out"]) for i in range(N_CORES)])
    return np.ascontiguousarray(
        out.reshape(256, L, C)).astype(np.float32)


# revision 7
# speedup vs baseline: 1.4084x; 1.4084x over previous
"""Trainium2 Bass kernel for the video-adapter module.

Computation (per clip of T=8 frames, H=W=14, C=768, CA=384):
  h  = fc1(x[:, 1:, :])                    # 768 -> 384
  h  = depthwise_conv3d(h, 3x3x3, pad 1)   # per-channel over (T, H, W)
  h  = fc2(h)                              # 384 -> 768
  out = x;  out[:, 1:, :] += h

Sharding: data-parallel over the clip axis — 8 cores x 4 clips each.
Inputs are replicated weights + a per-core x shard; no collectives.

Per-core dataflow (all token indices are clip-local, CLS rows kept
interleaved so every DMA/matmul tile is a clean 128-row block):
  DMA x (bf16, host-converted)  -> PE transpose -> xT (C-major)
  fc1 matmuls (W1T stationary)  -> h channel-major [128ch, 1576tok]
  depthwise conv = 27 fused scalar_tensor_tensor FMA taps (DVE/GPSIMD)
      acc += shift(h) * w[ch, tap]   with clipped valid ranges
  fc2 matmuls (acc slices stationary) -> psum token-major
  residual: out = psum + x (fused STT)  -> DMA out (f32)
"""

import os
import sys

sys.path.insert(0, "/opt/trn_rl_repo")

import numpy as np
import ml_dtypes

import concourse.bass as bass
import concourse.bacc as bacc
import concourse.mybir as mybir
import concourse.tile as tile
from concourse.ap import AP
from concourse.bass_utils import run_bass_kernel_spmd

BF16 = mybir.dt.bfloat16
F32 = mybir.dt.float32
NPBF16 = ml_dtypes.bfloat16
MULT = mybir.AluOpType.mult
ADD = mybir.AluOpType.add

N_CORES = 8
T = 8
L = 197            # tokens per frame incl CLS
GRID = 14          # H = W
C = 768
CA = 384
NCLIP = 4          # clips per core
CLIP_ROWS = T * L  # 1576
CORE_ROWS = NCLIP * CLIP_ROWS  # 6304
NBLK = 13          # 128-row blocks per clip (12 full + 40)
CHUNKS = [(0, 512), (512, 1024), (1024, 1536), (1536, 1576)]

# tap order: d = (dt+1)*9 + (dh+1)*3 + (dw+1); center = 13
TAPS = [(dt, dh, dw) for dt in (-1, 0, 1) for dh in (-1, 0, 1) for dw in (-1, 0, 1)]

# Engine per tap: "V" = vector (DVE), "G" = gpsimd. Tunable for balance.
TAP_ENGINE = {d: "V" for d in range(27)}

_CACHE = {}
LAST_RESULT = None
last_exec_time_ns = None


def _blk_rows(b):
    return 128 if b < NBLK - 1 else CLIP_ROWS - 128 * (NBLK - 1)


def _flat_ap(t, off, tcnt, n):
    """3-dim AP on a clip tile: partitions x (t frames, step 197) x (n flat
    tokens, step 1), starting at in-frame offset `off`."""
    full = t[:]
    return AP(full.tensor, full.offset + off, [list(full.ap[0]), [L, tcnt], [1, n]])


def _row_ap(t, off, tcnt, hcnt):
    """3-dim AP: partitions x (t frames, step 197) x (h rows, step 14)."""
    full = t[:]
    return AP(full.tensor, full.offset + off, [list(full.ap[0]), [L, tcnt], [GRID, hcnt]])


def _cls_ap(t):
    full = t[:]
    return AP(full.tensor, full.offset, [list(full.ap[0]), [L, T]])


def _build(has_b1, has_b2p, tap_engine):
    nc = bacc.Bacc("TRN2", target_bir_lowering=False, debug=False,
                   enable_asserts=False)

    x_d = nc.dram_tensor("x", [CORE_ROWS, C], BF16, kind="ExternalInput")
    w1t_d = nc.dram_tensor("w1t", [C, CA], BF16, kind="ExternalInput")
    w2t_d = nc.dram_tensor("w2t", [CA, C], BF16, kind="ExternalInput")
    cw_d = nc.dram_tensor("cw", [CA, 54], F32, kind="ExternalInput")
    id_d = nc.dram_tensor("ident", [128, 128], BF16, kind="ExternalInput")
    if has_b1:
        b1_d = nc.dram_tensor("b1", [CA, 1], F32, kind="ExternalInput")
    if has_b2p:
        b2p_d = nc.dram_tensor("b2p", [1, C], BF16, kind="ExternalInput")
        ones_d = nc.dram_tensor("ones", [1, 128], BF16, kind="ExternalInput")
    out_d = nc.dram_tensor("out", [CORE_ROWS, C], F32, kind="ExternalOutput")

    with tile.TileContext(nc) as tc:
        with (
            tc.tile_pool(name="const", bufs=1) as cp,
            tc.tile_pool(name="xbf", bufs=27) as xp,
            tc.tile_pool(name="xt", bufs=12) as xtp,
            tc.tile_pool(name="hcm", bufs=6) as hp,
            tc.tile_pool(name="acc", bufs=6) as accp,
            tc.tile_pool(name="osb", bufs=4) as op_,
            tc.tile_pool(name="pxt", bufs=2, space=bass.MemorySpace.PSUM) as pxt,
            tc.tile_pool(name="ph", bufs=2, space=bass.MemorySpace.PSUM) as php,
            tc.tile_pool(name="po", bufs=4, space=bass.MemorySpace.PSUM) as pop,
        ):
            w1 = []
            for g in range(6):
                t = cp.tile([128, CA], BF16, tag=f"w1_{g}")
                nc.sync.dma_start(t[:], w1t_d[128 * g:128 * (g + 1), :])
                w1.append(t)
            w2 = []
            for k in range(3):
                t = cp.tile([128, C], BF16, tag=f"w2_{k}")
                nc.sync.dma_start(t[:], w2t_d[128 * k:128 * (k + 1), :])
                w2.append(t)
            cw = []
            for m in range(3):
                t = cp.tile([128, 54], F32, tag=f"cw_{m}")
                nc.sync.dma_start(t[:], cw_d[128 * m:128 * (m + 1), :])
                cw.append(t)
            ident = cp.tile([128, 128], BF16, tag="ident")
            nc.sync.dma_start(ident[:], id_d[:, :])
            if has_b1:
                b1sb = []
                for m in range(3):
                    t = cp.tile([128, 1], F32, tag=f"b1_{m}")
                    nc.sync.dma_start(t[:], b1_d[128 * m:128 * (m + 1), :])
                    b1sb.append(t)
            if has_b2p:
                b2p = cp.tile([1, C], BF16, tag="b2p")
                nc.sync.dma_start(b2p[:], b2p_d[:, :])
                ones = cp.tile([1, 128], BF16, tag="ones")
                nc.sync.dma_start(ones[:], ones_d[:, :])

            for c in range(NCLIP):
                base = CLIP_ROWS * c
                # ---- load x blocks (bf16) ----
                xbf = []
                for b in range(NBLK):
                    nb = _blk_rows(b)
                    t = xp.tile([128, C], BF16, tag="xbf", name=f"xbf_{c}_{b}")
                    nc.sync.dma_start(
                        t[0:nb, :], x_d[base + 128 * b: base + 128 * b + nb, :])
                    xbf.append(t)

                # ---- transpose + fc1, chunk by chunk ----
                hcm = [hp.tile([128, CLIP_ROWS + 16], BF16, tag="hcm", name=f"hcm_{c}_{m}")
                       for m in range(3)]
                for (s0, s1) in CHUNKS:
                    w = s1 - s0
                    xt = [xtp.tile([128, 512], BF16, tag="xt", name=f"xt_{c}_{s0}_{g}")
                          for g in range(6)]
                    for g in range(6):
                        pst = pxt.tile([128, 512], BF16, tag="pxt", name=f"pxt_{c}_{s0}_{g}")
                        for bi in range(s0 // 128, (s1 + 127) // 128):
                            nb = min(_blk_rows(bi), s1 - 128 * bi)
                            nc.tensor.transpose(
                                pst[:, 128 * bi - s0:128 * bi - s0 + nb],
                                xbf[bi][0:nb, 128 * g:128 * (g + 1)],
                                ident[0:nb, 0:nb])
                        nc.scalar.copy(xt[g][:, 0:w], pst[:, 0:w])
                    for m in range(3):
                        psh = php.tile([128, 512], F32, tag="ph", name=f"ph_{c}_{s0}_{m}")
                        for g in range(6):
                            nc.tensor.matmul(
                                psh[:, 0:w],
                                w1[g][:, 128 * m:128 * (m + 1)],
                                xt[g][:, 0:w],
                                start=(g == 0), stop=(g == 5))
                        if has_b1:
                            nc.vector.tensor_scalar_add(
                                hcm[m][:, s0:s1], psh[:, 0:w], b1sb[m][:, 0:1])
                        else:
                            nc.scalar.copy(hcm[m][:, s0:s1], psh[:, 0:w])

                # ---- depthwise conv: fused FMA taps ----
                # Each tap runs on a (h,w)-flattened 3-dim AP (walrus caps
                # TensorScalarPtr at 2 free dims). For dw != 0 the flat run
                # wraps at row edges; the wrapped term is subtracted with a
                # small negated-weight fixup op (cols 27..53 of cw = -w).
                acc = [accp.tile([128, CLIP_ROWS], BF16, tag="acc", name=f"acc_{c}_{m}")
                       for m in range(3)]
                for m in range(3):
                    nc.gpsimd.memset(_cls_ap(acc[m]), 0.0)
                    nc.gpsimd.memset(hcm[m][:, CLIP_ROWS:], 0.0)
                    # center tap (13) must run first: it initializes acc
                    for d in [13] + [i for i in range(27) if i != 13]:
                        dt_, dh_, dw_ = TAPS[d]
                        t0, h0 = max(0, -dt_), max(0, -dh_)
                        tcn, hcn = T - abs(dt_), GRID - abs(dh_)
                        out_off = L * t0 + GRID * h0 + 1
                        in_off = (L * (t0 + dt_) + GRID * (h0 + dh_)
                                  + 1 + dw_)
                        n = GRID * hcn
                        oap = _flat_ap(acc[m], out_off, tcn, n)
                        iap = _flat_ap(hcm[m], in_off, tcn, n)
                        eng = nc.vector if tap_engine[d] == "V" else nc.gpsimd
                        if d == 13:
                            # center tap initializes the accumulator
                            nc.vector.tensor_scalar_mul(oap, iap, cw[m][:, d:d + 1])
                        else:
                            eng.scalar_tensor_tensor(
                                oap, iap, cw[m][:, d:d + 1], oap,
                                op0=MULT, op1=ADD)
                        if dw_ != 0:
                            wf = GRID - 1 if dw_ == 1 else 0
                            o2 = _row_ap(acc[m], L * t0 + GRID * h0 + wf + 1,
                                         tcn, hcn)
                            i2 = _row_ap(hcm[m],
                                         L * (t0 + dt_) + GRID * (h0 + dh_)
                                         + wf + dw_ + 1, tcn, hcn)
                            eng.scalar_tensor_tensor(
                                o2, i2, cw[m][:, 27 + d:28 + d], o2,
                                op0=MULT, op1=ADD)

                # ---- fc2 + residual + store ----
                for b in range(NBLK):
                    nb = _blk_rows(b)
                    pso = [pop.tile([128, CA], F32, tag="po", name=f"po_{c}_{b}_{nh}") for nh in range(2)]
                    for nh in range(2):
                        for k in range(3):
                            nc.tensor.matmul(
                                pso[nh][0:nb, :],
                                acc[k][:, 128 * b:128 * b + nb],
                                w2[k][:, CA * nh:CA * (nh + 1)],
                                start=(k == 0),
                                stop=(k == 2 and not has_b2p))
                        if has_b2p:
                            nc.tensor.matmul(
                                pso[nh][0:nb, :],
                                ones[0:1, 0:nb],
                                b2p[0:1, CA * nh:CA * (nh + 1)],
                                start=False, stop=True)
                    osb = op_.tile([128, C], F32, tag="osb", name=f"osb_{c}_{b}")
                    for nh in range(2):
                        nc.vector.scalar_tensor_tensor(
                            osb[0:nb, CA * nh:CA * (nh + 1)],
                            pso[nh][0:nb, :], 1.0,
                            xbf[b][0:nb, CA * nh:CA * (nh + 1)],
                            op0=MULT, op1=ADD)
                    nc.sync.dma_start(
                        out_d[base + 128 * b: base + 128 * b + nb, :],
                        osb[0:nb, :])

    nc.compile()
    return nc


def _get_nc(has_b1, has_b2p):
    key = (has_b1, has_b2p, tuple(sorted(TAP_ENGINE.items())))
    if key not in _CACHE:
        _CACHE[key] = _build(has_b1, has_b2p, TAP_ENGINE)
    return _CACHE[key]


def kernel(x, W1, b1, conv_w, conv_b, W2, b2, T=8):
    global LAST_RESULT, last_exec_time_ns
    x = np.asarray(x, dtype=np.float32)
    W1 = np.asarray(W1, dtype=np.float32)
    b1 = np.asarray(b1, dtype=np.float32)
    conv_w = np.asarray(conv_w, dtype=np.float32)
    conv_b = np.asarray(conv_b, dtype=np.float32)
    W2 = np.asarray(W2, dtype=np.float32)
    b2 = np.asarray(b2, dtype=np.float32)

    xs = np.ascontiguousarray(
        x.reshape(N_CORES, CORE_ROWS, C)).astype(NPBF16)
    w1t = np.ascontiguousarray(W1.T).astype(NPBF16)
    w2t = np.ascontiguousarray(W2.T).astype(NPBF16)
    cw27 = conv_w.reshape(CA, 27).astype(np.float32)
    cwf = np.ascontiguousarray(np.concatenate([cw27, -cw27], axis=1))
    b2p = (b2.astype(np.float64)
           + W2.astype(np.float64) @ conv_b.astype(np.float64))
    has_b1 = bool(np.any(b1 != 0.0))
    has_b2p = bool(np.any(b2p != 0.0))
    ident = np.eye(128, dtype=NPBF16)

    common = {
        "w1t": w1t,
        "w2t": w2t,
        "cw": cwf,
        "ident": ident,
    }
    if has_b1:
        common["b1"] = b1.reshape(CA, 1).astype(np.float32)
    if has_b2p:
        common["b2p"] = b2p.reshape(1, C).astype(NPBF16)
        common["ones"] = np.ones((1, 128), dtype=NPBF16)

    nc = _get_nc(has_b1, has_b2p)
    in_maps = [dict(common, x=xs[i]) for i in range(N_CORES)]
    trace = bool(int(os.environ.get("KERNEL_TRACE", "0")))
    res = run_bass_kernel_spmd(nc, in_maps, list(range(N_CORES)), trace=trace)
    LAST_RESULT = res
    last_exec_time_ns = res.exec_time_ns

    out = np.stack([np.asarray(res.results[i]["out"]) for i in range(N_CORES)])
    return np.ascontiguousarray(
        out.reshape(256, L, C)).astype(np.float32)


# revision 15
# speedup vs baseline: 2.2947x; 1.6293x over previous
"""Trainium2 Bass kernel for the video-adapter module.

Computation (per clip of T=8 frames, H=W=14, C=768, CA=384):
  h  = fc1(x[:, 1:, :])                    # 768 -> 384
  h  = depthwise_conv3d(h, 3x3x3, pad 1)   # per-channel over (T, H, W)
  h  = fc2(h)                              # 384 -> 768
  out = x;  out[:, 1:, :] += h

Sharding: data-parallel over the clip axis — 8 cores x 4 clips each.
Inputs are replicated weights + a per-core x shard; no collectives.

Per-core dataflow (all token indices are clip-local, CLS rows kept
interleaved so every DMA/matmul tile is a clean 128-row block):
  DMA x (bf16, host-converted)  -> PE transpose -> xT (C-major)
  fc1 matmuls (W1T stationary)  -> h channel-major [128ch, 1576tok]
  depthwise conv = 27 fused scalar_tensor_tensor FMA taps (DVE/GPSIMD)
      acc += shift(h) * w[ch, tap]   with clipped valid ranges
  fc2 matmuls (acc slices stationary) -> psum token-major
  residual: out = psum + x (fused STT)  -> DMA out (f32)
"""

import os
import sys

sys.path.insert(0, "/opt/trn_rl_repo")

import numpy as np
import ml_dtypes

import concourse.bass as bass
import concourse.bacc as bacc
import concourse.mybir as mybir
import concourse.tile as tile
from concourse.ap import AP
from concourse.bass_utils import run_bass_kernel_spmd

BF16 = mybir.dt.bfloat16
F32 = mybir.dt.float32
NPBF16 = ml_dtypes.bfloat16
MULT = mybir.AluOpType.mult
ADD = mybir.AluOpType.add

N_CORES = 8
T = 8
L = 197            # tokens per frame incl CLS
GRID = 14          # H = W
C = 768
CA = 384
NCLIP = 4          # clips per core
CLIP_ROWS = T * L  # 1576
CORE_ROWS = NCLIP * CLIP_ROWS  # 6304
NBLK = 13          # 128-row blocks per clip (12 full + 40)
CHUNKS = [(0, 512), (512, 1024), (1024, 1536), (1536, 1576)]

# tap order: d = (dt+1)*9 + (dh+1)*3 + (dw+1); center = 13
TAPS = [(dt, dh, dw) for dt in (-1, 0, 1) for dh in (-1, 0, 1) for dw in (-1, 0, 1)]

# Engine per tap: "P" = tensor engine (diag matmul, dw==0 only),
# "V" = vector (DVE), "G" = gpsimd. Tunable for balance.
def _default_tap_engine():
    te = {}
    for d, (dt_, dh_, dw_) in enumerate(TAPS):
        if dw_ == 0:
            te[d] = "P"
        else:
            te[d] = "V"
    return te

TAP_ENGINE = _default_tap_engine()
DW0_TAPS = [d for d, (dt_, dh_, dw_) in enumerate(TAPS) if dw_ == 0]

_CACHE = {}
LAST_RESULT = None
last_exec_time_ns = None


def _blk_rows(b):
    return 128 if b < NBLK - 1 else CLIP_ROWS - 128 * (NBLK - 1)


def _flat_ap(t, off, tcnt, n):
    """3-dim AP on a clip tile: partitions x (t frames, step 197) x (n flat
    tokens, step 1), starting at in-frame offset `off`."""
    full = t[:]
    return AP(full.tensor, full.offset + off, [list(full.ap[0]), [L, tcnt], [1, n]])


def _row_ap(t, off, tcnt, hcnt):
    """3-dim AP: partitions x (t frames, step 197) x (h rows, step 14)."""
    full = t[:]
    return AP(full.tensor, full.offset + off, [list(full.ap[0]), [L, tcnt], [GRID, hcnt]])


def _cls_ap(t):
    full = t[:]
    return AP(full.tensor, full.offset, [list(full.ap[0]), [L, T]])


def _build(has_b1, has_b2p, tap_engine, reps=1):
    nc = bacc.Bacc("TRN2", target_bir_lowering=False, debug=False,
                   enable_asserts=False)

    x_d = nc.dram_tensor("x", [CORE_ROWS, C], BF16, kind="ExternalInput")
    w1t_d = nc.dram_tensor("w1t", [C, CA], BF16, kind="ExternalInput")
    w2t_d = nc.dram_tensor("w2t", [CA, C], BF16, kind="ExternalInput")
    cw_d = nc.dram_tensor("cw", [CA, 54], F32, kind="ExternalInput")
    id_d = nc.dram_tensor("ident", [128, 128], BF16, kind="ExternalInput")
    pe_taps = [d for d in range(27) if tap_engine[d] == "P"]
    if pe_taps:
        assert all(TAPS[d][2] == 0 for d in pe_taps), "P taps must have dw==0"
        assert 13 in pe_taps, "center tap must be on PE to initialize psum"
        dw0_d = nc.dram_tensor("dw0", [len(DW0_TAPS), 3, 128, 128], BF16,
                               kind="ExternalInput")
    if has_b1:
        b1_d = nc.dram_tensor("b1", [CA, 1], F32, kind="ExternalInput")
    if has_b2p:
        b2p_d = nc.dram_tensor("b2p", [1, C], BF16, kind="ExternalInput")
        ones_d = nc.dram_tensor("ones", [1, 128], BF16, kind="ExternalInput")
    out_d = nc.dram_tensor("out", [CORE_ROWS, C], F32, kind="ExternalOutput")

    with tile.TileContext(nc) as tc:
        with (
            tc.tile_pool(name="const", bufs=1) as cp,
            tc.tile_pool(name="xbf", bufs=27) as xp,
            tc.tile_pool(name="xt", bufs=12) as xtp,
            tc.tile_pool(name="hcm", bufs=6) as hp,
            tc.tile_pool(name="acc", bufs=6) as accp,
            tc.tile_pool(name="osb", bufs=4) as op_,
            tc.tile_pool(name="pxt", bufs=2, space=bass.MemorySpace.PSUM) as pxt,
            tc.tile_pool(name="ph", bufs=3, space=bass.MemorySpace.PSUM) as php,
            tc.tile_pool(name="po", bufs=3, space=bass.MemorySpace.PSUM) as pop,
        ):
            w1 = []
            for g in range(6):
                t = cp.tile([128, CA], BF16, tag=f"w1_{g}")
                nc.sync.dma_start(t[:], w1t_d[128 * g:128 * (g + 1), :])
                w1.append(t)
            w2 = []
            for k in range(3):
                t = cp.tile([128, C], BF16, tag=f"w2_{k}")
                nc.sync.dma_start(t[:], w2t_d[128 * k:128 * (k + 1), :])
                w2.append(t)
            cw = []
            for m in range(3):
                t = cp.tile([128, 54], F32, tag=f"cw_{m}")
                nc.sync.dma_start(t[:], cw_d[128 * m:128 * (m + 1), :])
                cw.append(t)
            ident = cp.tile([128, 128], BF16, tag="ident")
            nc.sync.dma_start(ident[:], id_d[:, :])
            dw0sb = {}
            if pe_taps:
                for d in pe_taps:
                    i = DW0_TAPS.index(d)
                    for m in range(3):
                        t = cp.tile([128, 128], BF16, tag=f"dw0_{d}_{m}",
                                    name=f"dw0_{d}_{m}")
                        nc.sync.dma_start(t[:], dw0_d[i, m])
                        dw0sb[(d, m)] = t
            if has_b1:
                b1sb = []
                for m in range(3):
                    t = cp.tile([128, 1], F32, tag=f"b1_{m}")
                    nc.sync.dma_start(t[:], b1_d[128 * m:128 * (m + 1), :])
                    b1sb.append(t)
            if has_b2p:
                b2p = cp.tile([1, C], BF16, tag="b2p")
                nc.sync.dma_start(b2p[:], b2p_d[:, :])
                ones = cp.tile([1, 128], BF16, tag="ones")
                nc.sync.dma_start(ones[:], ones_d[:, :])

            for rep in range(reps):
              for c in range(NCLIP):
                base = CLIP_ROWS * c
                # ---- load x blocks (bf16) ----
                xbf = []
                for b in range(NBLK):
                    nb = _blk_rows(b)
                    t = xp.tile([128, C], BF16, tag="xbf", name=f"xbf_{c}_{b}")
                    nc.sync.dma_start(
                        t[0:nb, :], x_d[base + 128 * b: base + 128 * b + nb, :])
                    xbf.append(t)

                # ---- transpose + fc1, chunk by chunk ----
                hcm = [hp.tile([128, CLIP_ROWS + 16], BF16, tag="hcm", name=f"hcm_{c}_{m}")
                       for m in range(3)]
                for (s0, s1) in CHUNKS:
                    w = s1 - s0
                    xt = [xtp.tile([128, 512], BF16, tag="xt", name=f"xt_{c}_{s0}_{g}")
                          for g in range(6)]
                    for g in range(6):
                        pst = pxt.tile([128, 512], BF16, tag="pxt", name=f"pxt_{c}_{s0}_{g}")
                        for bi in range(s0 // 128, (s1 + 127) // 128):
                            nb = min(_blk_rows(bi), s1 - 128 * bi)
                            nc.tensor.transpose(
                                pst[:, 128 * bi - s0:128 * bi - s0 + nb],
                                xbf[bi][0:nb, 128 * g:128 * (g + 1)],
                                ident[0:nb, 0:nb])
                        nc.scalar.copy(xt[g][:, 0:w], pst[:, 0:w])
                    for m in range(3):
                        psh = php.tile([128, 512], F32, tag="ph", name=f"ph_{c}_{s0}_{m}")
                        for g in range(6):
                            nc.tensor.matmul(
                                psh[:, 0:w],
                                w1[g][:, 128 * m:128 * (m + 1)],
                                xt[g][:, 0:w],
                                start=(g == 0), stop=(g == 5))
                        if has_b1:
                            nc.vector.tensor_scalar_add(
                                hcm[m][:, s0:s1], psh[:, 0:w], b1sb[m][:, 0:1])
                        else:
                            nc.scalar.copy(hcm[m][:, s0:s1], psh[:, 0:w])

                # ---- depthwise conv ----
                # dw==0 taps can run on the tensor engine as diag-weight
                # matmuls accumulating in PSUM (per 2-frame quarter, one
                # PSUM bank); acc is then initialized by a PSUM->SBUF copy.
                # dw!=0 taps run as fused scalar_tensor_tensor FMAs on
                # DVE/GPSIMD over (h,w)-flattened 3-dim APs (walrus caps
                # TensorScalarPtr at 2 free dims); the flat run wraps at row
                # edges, and the wrapped term is subtracted with a negated-
                # weight fixup op (cols 27..53 of cw = -w).
                acc = [accp.tile([128, CLIP_ROWS], BF16, tag="acc", name=f"acc_{c}_{m}")
                       for m in range(3)]
                for m in range(3):
                    nc.gpsimd.memset(_cls_ap(acc[m]), 0.0)
                    nc.gpsimd.memset(hcm[m][:, CLIP_ROWS:], 0.0)
                    if pe_taps:
                        for q4 in range(4):
                            pcv = php.tile([128, 2 * L], F32, tag="ph",
                                           name=f"pcv_{c}_{m}_{q4}")
                            mms = []
                            for d in [13] + [i for i in pe_taps if i != 13]:
                                dt_, dh_, _ = TAPS[d]
                                t0, h0 = max(0, -dt_), max(0, -dh_)
                                tcn, hcn = T - abs(dt_), GRID - abs(dh_)
                                a = max(t0, 2 * q4)
                                b_ = min(t0 + tcn, 2 * q4 + 2)
                                if a >= b_:
                                    continue
                                n = GRID * hcn
                                oap = _flat_ap(pcv, L * (a - 2 * q4)
                                               + GRID * h0 + 1, b_ - a, n)
                                iap = _flat_ap(hcm[m], L * (a + dt_)
                                               + GRID * (h0 + dh_) + 1,
                                               b_ - a, n)
                                mms.append((dw0sb[(d, m)], iap, oap))
                            for i, (lh, rh, oa) in enumerate(mms):
                                nc.tensor.matmul(oa, lh[:], rh,
                                                 start=(i == 0),
                                                 stop=(i == len(mms) - 1))
                            # init acc quarter from psum (f32 -> bf16)
                            nc.scalar.copy(
                                _flat_ap(acc[m], L * 2 * q4 + 1, 2, 196),
                                _flat_ap(pcv, 1, 2, 196))
                    for d in [13] + [i for i in range(27) if i != 13]:
                        if tap_engine[d] == "P":
                            continue
                        dt_, dh_, dw_ = TAPS[d]
                        t0, h0 = max(0, -dt_), max(0, -dh_)
                        tcn, hcn = T - abs(dt_), GRID - abs(dh_)
                        out_off = L * t0 + GRID * h0 + 1
                        in_off = (L * (t0 + dt_) + GRID * (h0 + dh_)
                                  + 1 + dw_)
                        n = GRID * hcn
                        oap = _flat_ap(acc[m], out_off, tcn, n)
                        iap = _flat_ap(hcm[m], in_off, tcn, n)
                        eng = nc.vector if tap_engine[d] == "V" else nc.gpsimd
                        if d == 13 and not pe_taps:
                            # center tap initializes the accumulator
                            nc.vector.tensor_scalar_mul(oap, iap, cw[m][:, d:d + 1])
                        else:
                            eng.scalar_tensor_tensor(
                                oap, iap, cw[m][:, d:d + 1], oap,
                                op0=MULT, op1=ADD)
                        if dw_ != 0:
                            wf = GRID - 1 if dw_ == 1 else 0
                            o2 = _row_ap(acc[m], L * t0 + GRID * h0 + wf + 1,
                                         tcn, hcn)
                            i2 = _row_ap(hcm[m],
                                         L * (t0 + dt_) + GRID * (h0 + dh_)
                                         + wf + dw_ + 1, tcn, hcn)
                            eng.scalar_tensor_tensor(
                                o2, i2, cw[m][:, 27 + d:28 + d], o2,
                                op0=MULT, op1=ADD)

                # ---- fc2 + fused residual (identity matmul) + store ----
                for b in range(NBLK):
                    nb = _blk_rows(b)
                    pso = [pop.tile([128, CA], F32, tag="po", name=f"po_{c}_{b}_{nh}") for nh in range(2)]
                    for nh in range(2):
                        for k in range(3):
                            nc.tensor.matmul(
                                pso[nh][0:nb, :],
                                acc[k][:, 128 * b:128 * b + nb],
                                w2[k][:, CA * nh:CA * (nh + 1)],
                                start=(k == 0), stop=False)
                        if has_b2p:
                            nc.tensor.matmul(
                                pso[nh][0:nb, :],
                                ones[0:1, 0:nb],
                                b2p[0:1, CA * nh:CA * (nh + 1)],
                                start=False, stop=False)
                        # residual: psum += I.T @ x  (x lands in psum exactly)
                        nc.tensor.matmul(
                            pso[nh][0:nb, :],
                            ident[0:nb, 0:nb],
                            xbf[b][0:nb, CA * nh:CA * (nh + 1)],
                            start=False, stop=True)
                    osb = op_.tile([128, C], F32, tag="osb", name=f"osb_{c}_{b}")
                    for nh in range(2):
                        nc.scalar.copy(osb[0:nb, CA * nh:CA * (nh + 1)],
                                       pso[nh][0:nb, :])
                    nc.sync.dma_start(
                        out_d[base + 128 * b: base + 128 * b + nb, :],
                        osb[0:nb, :])

    nc.compile()
    return nc


def _get_nc(has_b1, has_b2p):
    key = (has_b1, has_b2p, tuple(sorted(TAP_ENGINE.items())))
    if key not in _CACHE:
        _CACHE[key] = _build(has_b1, has_b2p, TAP_ENGINE)
    return _CACHE[key]


def kernel(x, W1, b1, conv_w, conv_b, W2, b2, T=8):
    global LAST_RESULT, last_exec_time_ns
    x = np.asarray(x, dtype=np.float32)
    W1 = np.asarray(W1, dtype=np.float32)
    b1 = np.asarray(b1, dtype=np.float32)
    conv_w = np.asarray(conv_w, dtype=np.float32)
    conv_b = np.asarray(conv_b, dtype=np.float32)
    W2 = np.asarray(W2, dtype=np.float32)
    b2 = np.asarray(b2, dtype=np.float32)

    xs = np.ascontiguousarray(
        x.reshape(N_CORES, CORE_ROWS, C)).astype(NPBF16)
    w1t = np.ascontiguousarray(W1.T).astype(NPBF16)
    w2t = np.ascontiguousarray(W2.T).astype(NPBF16)
    cw27 = conv_w.reshape(CA, 27).astype(np.float32)
    cwf = np.ascontiguousarray(np.concatenate([cw27, -cw27], axis=1))
    b2p = (b2.astype(np.float64)
           + W2.astype(np.float64) @ conv_b.astype(np.float64))
    has_b1 = bool(np.any(b1 != 0.0))
    has_b2p = bool(np.any(b2p != 0.0))
    ident = np.eye(128, dtype=NPBF16)

    common = {
        "w1t": w1t,
        "w2t": w2t,
        "cw": cwf,
        "ident": ident,
    }
    if any(v == "P" for v in TAP_ENGINE.values()):
        dw0 = np.zeros((len(DW0_TAPS), 3, 128, 128), dtype=NPBF16)
        for i, d in enumerate(DW0_TAPS):
            for m in range(3):
                np.fill_diagonal(dw0[i, m], cw27[128 * m:128 * (m + 1), d])
        common["dw0"] = dw0
    if has_b1:
        common["b1"] = b1.reshape(CA, 1).astype(np.float32)
    if has_b2p:
        common["b2p"] = b2p.reshape(1, C).astype(NPBF16)
        common["ones"] = np.ones((1, 128), dtype=NPBF16)

    nc = _get_nc(has_b1, has_b2p)
    in_maps = [dict(common, x=xs[i]) for i in range(N_CORES)]
    trace = bool(int(os.environ.get("KERNEL_TRACE", "0")))
    res = run_bass_kernel_spmd(nc, in_maps, list(range(N_CORES)), trace=trace)
    LAST_RESULT = res
    last_exec_time_ns = res.exec_time_ns

    out = np.stack([np.asarray(res.results[i]["out"]) for i in range(N_CORES)])
    return np.ascontiguousarray(
        out.reshape(256, L, C)).astype(np.float32)


# revision 21
# speedup vs baseline: 24335.6563x; 10605.1735x over previous
"""Trainium2 Bass kernel for the video-adapter module.

Computation (per clip of T=8 frames, H=W=14, C=768, CA=384):
  h  = fc1(x[:, 1:, :])                    # 768 -> 384
  h  = depthwise_conv3d(h, 3x3x3, pad 1)   # per-channel over (T, H, W)
  h  = fc2(h)                              # 384 -> 768
  out = x;  out[:, 1:, :] += h

Sharding: data-parallel over the clip axis — 8 cores x 4 clips each.
Inputs are replicated weights + a per-core x shard; no collectives.

Per-core dataflow (all token indices are clip-local, CLS rows kept
interleaved so every DMA/matmul tile is a clean 128-row block):
  x (bf16, host-converted): loaded token-major for the residual AND
      channel-major via DMA-xbar transpose (dma_start_transpose) for fc1
  fc1 matmuls (W1T stationary)  -> h channel-major [128ch, 1576tok]
  depthwise conv, split across engines (TAP_ENGINE):
    - "P" taps: diagonal-weight matmuls accumulating in PSUM per 2-frame
      quarter; acc is then initialized by an ACT PSUM->SBUF copy
    - "V"/"G" taps: fused scalar_tensor_tensor FMAs over (h,w)-flattened
      3-dim APs; row-edge wrap of the flat run is corrected by a small
      negated-weight fixup op (cols 27..53 of cw = -w)
  fc2 matmuls (acc slices stationary) -> psum token-major
  residual: out = psum + x (fused STT on DVE)  -> DMA out (f32)
"""

import os
import sys

sys.path.insert(0, "/opt/trn_rl_repo")

import numpy as np
import ml_dtypes

import concourse.bass as bass
import concourse.bacc as bacc
import concourse.mybir as mybir
import concourse.tile as tile
from concourse.ap import AP
from concourse.bass_utils import run_bass_kernel_spmd

BF16 = mybir.dt.bfloat16
F32 = mybir.dt.float32
NPBF16 = ml_dtypes.bfloat16
MULT = mybir.AluOpType.mult
ADD = mybir.AluOpType.add

N_CORES = 8
T = 8
L = 197            # tokens per frame incl CLS
GRID = 14          # H = W
C = 768
CA = 384
NCLIP = 4          # clips per core
CLIP_ROWS = T * L  # 1576
CORE_ROWS = NCLIP * CLIP_ROWS  # 6304
NBLK = 13          # 128-row blocks per clip (12 full + 40)
CHUNKS = [(0, 512), (512, 1024), (1024, 1536), (1536, 1576)]

# tap order: d = (dt+1)*9 + (dh+1)*3 + (dw+1); center = 13
TAPS = [(dt, dh, dw) for dt in (-1, 0, 1) for dh in (-1, 0, 1) for dw in (-1, 0, 1)]

# Engine per tap: "P" = tensor engine (diag matmul, dw==0 only),
# "V" = vector (DVE), "G" = gpsimd. Tunable for balance.
def _default_tap_engine():
    te = {}
    for d, (dt_, dh_, dw_) in enumerate(TAPS):
        if dw_ == 0:
            te[d] = "P"
        else:
            te[d] = "V"
    return te

TAP_ENGINE = _default_tap_engine()
DW0_TAPS = [d for d, (dt_, dh_, dw_) in enumerate(TAPS) if dw_ == 0]

# True: transpose x via the DMA xbar (DRAM -> SBUF, bf16) instead of PE
# transposes + ACT psum copies.
USE_DMA_T = True

# "stt": residual add on DVE (scalar_tensor_tensor from psum; needs a second,
# token-major read of x). "mm": fold the residual into fc2's psum via an
# identity matmul. "xtmm": rebuild the residual from the already-transposed
# xT tiles (psum += xT.T @ I) — x is then read from DRAM only once, cutting
# ~25% of HBM traffic; measured ~19% faster than "stt".
RESID = "xtmm"

# PSUM pool sizes (banks: pxt + ph + po <= 8)
PSUM_BUFS = (2, 3, 3)

_CACHE = {}
LAST_RESULT = None
last_exec_time_ns = None


def _blk_rows(b):
    return 128 if b < NBLK - 1 else CLIP_ROWS - 128 * (NBLK - 1)


def _flat_ap(t, off, tcnt, n):
    """3-dim AP on a clip tile: partitions x (t frames, step 197) x (n flat
    tokens, step 1), starting at in-frame offset `off`."""
    full = t[:]
    return AP(full.tensor, full.offset + off, [list(full.ap[0]), [L, tcnt], [1, n]])


def _row_ap(t, off, tcnt, hcnt):
    """3-dim AP: partitions x (t frames, step 197) x (h rows, step 14)."""
    full = t[:]
    return AP(full.tensor, full.offset + off, [list(full.ap[0]), [L, tcnt], [GRID, hcnt]])


def _cls_ap(t):
    full = t[:]
    return AP(full.tensor, full.offset, [list(full.ap[0]), [L, T]])


def _build(has_b1, has_b2p, tap_engine, reps=1, use_dma_t=None, resid=None,
           psum_bufs=None):
    if use_dma_t is None:
        use_dma_t = USE_DMA_T
    if resid is None:
        resid = RESID
    if psum_bufs is None:
        psum_bufs = PSUM_BUFS
    nc = bacc.Bacc("TRN2", target_bir_lowering=False, debug=False,
                   enable_asserts=False)

    x_d = nc.dram_tensor("x", [CORE_ROWS, C], BF16, kind="ExternalInput")
    w1t_d = nc.dram_tensor("w1t", [C, CA], BF16, kind="ExternalInput")
    w2t_d = nc.dram_tensor("w2t", [CA, C], BF16, kind="ExternalInput")
    cw_d = nc.dram_tensor("cw", [CA, 54], F32, kind="ExternalInput")
    id_d = nc.dram_tensor("ident", [128, 128], BF16, kind="ExternalInput")
    pe_taps = [d for d in range(27) if tap_engine[d] == "P"]
    if pe_taps:
        assert 13 in pe_taps, "center tap must be on PE to initialize psum"
        dw0_d = nc.dram_tensor("dw0", [27, 3, 128, 128], BF16,
                               kind="ExternalInput")
    if has_b1:
        b1_d = nc.dram_tensor("b1", [CA, 1], F32, kind="ExternalInput")
    if has_b2p:
        b2p_d = nc.dram_tensor("b2p", [1, C], BF16, kind="ExternalInput")
        ones_d = nc.dram_tensor("ones", [1, 128], BF16, kind="ExternalInput")
    out_d = nc.dram_tensor("out", [CORE_ROWS, C], F32, kind="ExternalOutput")

    with tile.TileContext(nc) as tc:
        with (
            tc.tile_pool(name="const", bufs=1) as cp,
            tc.tile_pool(name="xbf", bufs=27) as xp,
            tc.tile_pool(name="xt", bufs=(48 if resid == "xtmm" else 12)) as xtp,
            tc.tile_pool(name="hcm", bufs=6) as hp,
            tc.tile_pool(name="acc", bufs=6) as accp,
            tc.tile_pool(name="osb", bufs=4) as op_,
            tc.tile_pool(name="pxt", bufs=psum_bufs[0], space=bass.MemorySpace.PSUM) as pxt,
            tc.tile_pool(name="ph", bufs=psum_bufs[1], space=bass.MemorySpace.PSUM) as php,
            tc.tile_pool(name="po", bufs=psum_bufs[2], space=bass.MemorySpace.PSUM) as pop,
        ):
            pcv_pool = php
            w1 = []
            for g in range(6):
                t = cp.tile([128, CA], BF16, tag=f"w1_{g}")
                nc.sync.dma_start(t[:], w1t_d[128 * g:128 * (g + 1), :])
                w1.append(t)
            w2 = []
            for k in range(3):
                t = cp.tile([128, C], BF16, tag=f"w2_{k}")
                nc.sync.dma_start(t[:], w2t_d[128 * k:128 * (k + 1), :])
                w2.append(t)
            cw = []
            for m in range(3):
                t = cp.tile([128, 54], F32, tag=f"cw_{m}")
                nc.sync.dma_start(t[:], cw_d[128 * m:128 * (m + 1), :])
                cw.append(t)
            ident = cp.tile([128, 128], BF16, tag="ident")
            nc.sync.dma_start(ident[:], id_d[:, :])
            dw0sb = {}
            if pe_taps:
                for d in pe_taps:
                    for m in range(3):
                        t = cp.tile([128, 128], BF16, tag=f"dw0_{d}_{m}",
                                    name=f"dw0_{d}_{m}")
                        nc.sync.dma_start(t[:], dw0_d[d, m])
                        dw0sb[(d, m)] = t
            if has_b1:
                b1sb = []
                for m in range(3):
                    t = cp.tile([128, 1], F32, tag=f"b1_{m}")
                    nc.sync.dma_start(t[:], b1_d[128 * m:128 * (m + 1), :])
                    b1sb.append(t)
            if has_b2p:
                b2p = cp.tile([1, C], BF16, tag="b2p")
                nc.sync.dma_start(b2p[:], b2p_d[:, :])
                ones = cp.tile([1, 128], BF16, tag="ones")
                nc.sync.dma_start(ones[:], ones_d[:, :])

            for rep in range(reps):
              for c in range(NCLIP):
                base = CLIP_ROWS * c
                # ---- load x blocks (bf16) ----
                # With resid == "xtmm" the residual is rebuilt from the
                # transposed xT tiles, so only the 40-row tail block (whose
                # transpose can't use the 16-row-quantized DMA xbar) is
                # loaded token-major.
                xbf = []
                for b in range(NBLK):
                    nb = _blk_rows(b)
                    if resid == "xtmm" and b < NBLK - 1:
                        xbf.append(None)
                        continue
                    t = xp.tile([128, C], BF16, tag="xbf", name=f"xbf_{c}_{b}")
                    nc.sync.dma_start(
                        t[0:nb, :], x_d[base + 128 * b: base + 128 * b + nb, :])
                    xbf.append(t)

                # ---- transpose + fc1, chunk by chunk ----
                hcm = [hp.tile([128, CLIP_ROWS + 16], BF16, tag="hcm", name=f"hcm_{c}_{m}")
                       for m in range(3)]
                xts = []
                for (s0, s1) in CHUNKS:
                    w = s1 - s0
                    xt = [xtp.tile([128, 512], BF16, tag="xt", name=f"xt_{c}_{s0}_{g}")
                          for g in range(6)]
                    xts.append(xt)
                    for g in range(6):
                        if use_dma_t and w % 16 == 0:
                            nc.sync.dma_start_transpose(
                                out=xt[g][:, 0:w],
                                in_=x_d[base + s0:base + s1,
                                        128 * g:128 * (g + 1)])
                            continue
                        pst = pxt.tile([128, 512], BF16, tag="pxt", name=f"pxt_{c}_{s0}_{g}")
                        for bi in range(s0 // 128, (s1 + 127) // 128):
                            nb = min(_blk_rows(bi), s1 - 128 * bi)
                            nc.tensor.transpose(
                                pst[:, 128 * bi - s0:128 * bi - s0 + nb],
                                xbf[bi][0:nb, 128 * g:128 * (g + 1)],
                                ident[0:nb, 0:nb])
                        nc.scalar.copy(xt[g][:, 0:w], pst[:, 0:w])
                    for m in range(3):
                        psh = php.tile([128, 512], F32, tag="ph", name=f"ph_{c}_{s0}_{m}")
                        for g in range(6):
                            nc.tensor.matmul(
                                psh[:, 0:w],
                                w1[g][:, 128 * m:128 * (m + 1)],
                                xt[g][:, 0:w],
                                start=(g == 0), stop=(g == 5))
                        if has_b1:
                            nc.vector.tensor_scalar_add(
                                hcm[m][:, s0:s1], psh[:, 0:w], b1sb[m][:, 0:1])
                        else:
                            nc.scalar.copy(hcm[m][:, s0:s1], psh[:, 0:w])

                # ---- depthwise conv ----
                # dw==0 taps can run on the tensor engine as diag-weight
                # matmuls accumulating in PSUM (per 2-frame quarter, one
                # PSUM bank); acc is then initialized by a PSUM->SBUF copy.
                # dw!=0 taps run as fused scalar_tensor_tensor FMAs on
                # DVE/GPSIMD over (h,w)-flattened 3-dim APs (walrus caps
                # TensorScalarPtr at 2 free dims); the flat run wraps at row
                # edges, and the wrapped term is subtracted with a negated-
                # weight fixup op (cols 27..53 of cw = -w).
                acc = [accp.tile([128, CLIP_ROWS], BF16, tag="acc", name=f"acc_{c}_{m}")
                       for m in range(3)]
                for m in range(3):
                    nc.gpsimd.memset(_cls_ap(acc[m]), 0.0)
                    nc.gpsimd.memset(hcm[m][:, CLIP_ROWS:], 0.0)
                    if pe_taps:
                        for q4 in range(4):
                            pcv = pcv_pool.tile([128, 2 * L], F32, tag="ph",
                                           name=f"pcv_{c}_{m}_{q4}")
                            mms = []
                            for d in [13] + [i for i in pe_taps if i != 13]:
                                dt_, dh_, dw_ = TAPS[d]
                                t0, h0 = max(0, -dt_), max(0, -dh_)
                                tcn, hcn = T - abs(dt_), GRID - abs(dh_)
                                a = max(t0, 2 * q4)
                                b_ = min(t0 + tcn, 2 * q4 + 2)
                                if a >= b_:
                                    continue
                                n = GRID * hcn
                                oap = _flat_ap(pcv, L * (a - 2 * q4)
                                               + GRID * h0 + 1, b_ - a, n)
                                iap = _flat_ap(hcm[m], L * (a + dt_)
                                               + GRID * (h0 + dh_) + 1 + dw_,
                                               b_ - a, n)
                                mms.append((dw0sb[(d, m)], iap, oap))
                            for i, (lh, rh, oa) in enumerate(mms):
                                nc.tensor.matmul(oa, lh[:], rh,
                                                 start=(i == 0),
                                                 stop=(i == len(mms) - 1))
                            # init acc quarter from psum (f32 -> bf16)
                            nc.scalar.copy(
                                _flat_ap(acc[m], L * 2 * q4 + 1, 2, 196),
                                _flat_ap(pcv, 1, 2, 196))
                    for d in [13] + [i for i in range(27) if i != 13]:
                        dt_, dh_, dw_ = TAPS[d]
                        if tap_engine[d] == "P":
                            if dw_ != 0:
                                # wrap fixup for a PE flat tap (after init copy)
                                t0, h0 = max(0, -dt_), max(0, -dh_)
                                tcn, hcn = T - abs(dt_), GRID - abs(dh_)
                                wf = GRID - 1 if dw_ == 1 else 0
                                o2 = _row_ap(acc[m], L * t0 + GRID * h0 + wf + 1,
                                             tcn, hcn)
                                i2 = _row_ap(hcm[m],
                                             L * (t0 + dt_) + GRID * (h0 + dh_)
                                             + wf + dw_ + 1, tcn, hcn)
                                nc.vector.scalar_tensor_tensor(
                                    o2, i2, cw[m][:, 27 + d:28 + d], o2,
                                    op0=MULT, op1=ADD)
                            continue
                        t0, h0 = max(0, -dt_), max(0, -dh_)
                        tcn, hcn = T - abs(dt_), GRID - abs(dh_)
                        out_off = L * t0 + GRID * h0 + 1
                        in_off = (L * (t0 + dt_) + GRID * (h0 + dh_)
                                  + 1 + dw_)
                        n = GRID * hcn
                        oap = _flat_ap(acc[m], out_off, tcn, n)
                        iap = _flat_ap(hcm[m], in_off, tcn, n)
                        eng = nc.vector if tap_engine[d] == "V" else nc.gpsimd
                        if d == 13 and not pe_taps:
                            # center tap initializes the accumulator
                            nc.vector.tensor_scalar_mul(oap, iap, cw[m][:, d:d + 1])
                        else:
                            eng.scalar_tensor_tensor(
                                oap, iap, cw[m][:, d:d + 1], oap,
                                op0=MULT, op1=ADD)
                        if dw_ != 0:
                            wf = GRID - 1 if dw_ == 1 else 0
                            o2 = _row_ap(acc[m], L * t0 + GRID * h0 + wf + 1,
                                         tcn, hcn)
                            i2 = _row_ap(hcm[m],
                                         L * (t0 + dt_) + GRID * (h0 + dh_)
                                         + wf + dw_ + 1, tcn, hcn)
                            eng.scalar_tensor_tensor(
                                o2, i2, cw[m][:, 27 + d:28 + d], o2,
                                op0=MULT, op1=ADD)

                # ---- fc2 + fused residual (identity matmul) + store ----
                for b in range(NBLK):
                    nb = _blk_rows(b)
                    pso = [pop.tile([128, CA], F32, tag="po", name=f"po_{c}_{b}_{nh}") for nh in range(2)]
                    for nh in range(2):
                        for k in range(3):
                            nc.tensor.matmul(
                                pso[nh][0:nb, :],
                                acc[k][:, 128 * b:128 * b + nb],
                                w2[k][:, CA * nh:CA * (nh + 1)],
                                start=(k == 0),
                                stop=(k == 2 and resid == "stt" and not has_b2p))
                        if has_b2p:
                            nc.tensor.matmul(
                                pso[nh][0:nb, :],
                                ones[0:1, 0:nb],
                                b2p[0:1, CA * nh:CA * (nh + 1)],
                                start=False, stop=(resid == "stt"))
                        if resid == "mm":
                            # residual: psum += I.T @ x
                            nc.tensor.matmul(
                                pso[nh][0:nb, :],
                                ident[0:nb, 0:nb],
                                xbf[b][0:nb, CA * nh:CA * (nh + 1)],
                                start=False, stop=True)
                        elif resid == "xtmm":
                            # residual from transposed x: psum_cols += xT.T @ I
                            xt_c = xts[b // 4]
                            off = 128 * b - 512 * (b // 4)
                            for j in range(3):
                                g = 3 * nh + j
                                nc.tensor.matmul(
                                    pso[nh][0:nb, 128 * j:128 * (j + 1)],
                                    xt_c[g][:, off:off + nb],
                                    ident[0:128, 0:128],
                                    start=False, stop=(j == 2))
                    osb = op_.tile([128, C], F32, tag="osb", name=f"osb_{c}_{b}")
                    for nh in range(2):
                        if resid in ("mm", "xtmm"):
                            nc.scalar.copy(osb[0:nb, CA * nh:CA * (nh + 1)],
                                           pso[nh][0:nb, :])
                        else:
                            nc.vector.scalar_tensor_tensor(
                                osb[0:nb, CA * nh:CA * (nh + 1)],
                                pso[nh][0:nb, :], 1.0,
                                xbf[b][0:nb, CA * nh:CA * (nh + 1)],
                                op0=MULT, op1=ADD)
                    nc.sync.dma_start(
                        out_d[base + 128 * b: base + 128 * b + nb, :],
                        osb[0:nb, :])

    nc.compile()
    return nc


def _get_nc(has_b1, has_b2p):
    key = (has_b1, has_b2p, tuple(sorted(TAP_ENGINE.items())), USE_DMA_T,
           RESID, PSUM_BUFS)
    if key not in _CACHE:
        _CACHE[key] = _build(has_b1, has_b2p, TAP_ENGINE)
    return _CACHE[key]


def kernel(x, W1, b1, conv_w, conv_b, W2, b2, T=8):
    global LAST_RESULT, last_exec_time_ns
    x = np.asarray(x, dtype=np.float32)
    W1 = np.asarray(W1, dtype=np.float32)
    b1 = np.asarray(b1, dtype=np.float32)
    conv_w = np.asarray(conv_w, dtype=np.float32)
    conv_b = np.asarray(conv_b, dtype=np.float32)
    W2 = np.asarray(W2, dtype=np.float32)
    b2 = np.asarray(b2, dtype=np.float32)

    xs = np.ascontiguousarray(
        x.reshape(N_CORES, CORE_ROWS, C)).astype(NPBF16)
    w1t = np.ascontiguousarray(W1.T).astype(NPBF16)
    w2t = np.ascontiguousarray(W2.T).astype(NPBF16)
    cw27 = conv_w.reshape(CA, 27).astype(np.float32)
    cwf = np.ascontiguousarray(np.concatenate([cw27, -cw27], axis=1))
    b2p = (b2.astype(np.float64)
           + W2.astype(np.float64) @ conv_b.astype(np.float64))
    has_b1 = bool(np.any(b1 != 0.0))
    has_b2p = bool(np.any(b2p != 0.0))
    ident = np.eye(128, dtype=NPBF16)

    common = {
        "w1t": w1t,
        "w2t": w2t,
        "cw": cwf,
        "ident": ident,
    }
    if any(v == "P" for v in TAP_ENGINE.values()):
        dw0 = np.zeros((27, 3, 128, 128), dtype=NPBF16)
        for d in range(27):
            for m in range(3):
                np.fill_diagonal(dw0[d, m], cw27[128 * m:128 * (m + 1), d])
        common["dw0"] = dw0
    if has_b1:
        common["b1"] = b1.reshape(CA, 1).astype(np.float32)
    if has_b2p:
        common["b2p"] = b2p.reshape(1, C).astype(NPBF16)
        common["ones"] = np.ones((1, 128), dtype=NPBF16)

    nc = _get_nc(has_b1, has_b2p)
    in_maps = [dict(common, x=xs[i]) for i in range(N_CORES)]
    trace = bool(int(os.environ.get("KERNEL_TRACE", "0")))
    res = run_bass_kernel_spmd(nc, in_maps, list(range(N_CORES)), trace=trace)
    LAST_RESULT = res
    last_exec_time_ns = res.exec_time_ns

    out = np.stack([np.asarray(res.results[i]["out"]) for i in range(N_CORES)])
    return np.ascontiguousarray(
        out.reshape(256, L, C)).astype(np.float32)

